# revision 1
# baseline (speedup 1.0000x reference)
"""Trainium2 Bass kernel for pre-LN multi-head attention (B=2, S=2048, H=1024, 16 heads).

Sharding: 8 cores = 2 batches x 4 query-blocks of 512 rows. Each core:
  - LayerNorm of its whole batch (stats via ones-matmul on transposed x)
  - K,V projections for the whole batch (duplicated across the 4 cores of a
    batch; avoids all collectives), Q projection for its own 512 rows
  - attention (scores^T dataflow: softmax denominator via an appended ones
    column on V), output projection + residual for its own rows.
Host reassembles the 8 disjoint [512, 1024] output slices.
"""

import sys
import numpy as np
from contextlib import ExitStack

sys.path.insert(0, "/opt/trn_rl_repo")

import concourse.bass as bass  # noqa: E402
import concourse.bacc as bacc  # noqa: E402
import concourse.tile as tile  # noqa: E402
from concourse import mybir  # noqa: E402

B, S, H = 2, 2048, 1024
HEADS, HD = 16, 64
NCORES = 8
SQ = 512          # query rows per core
HT = H // 128     # 8 hidden tiles
PAIRS = HEADS // 2
KCH = S // 128    # 16 key chunks of 128
F32 = mybir.dt.float32
F32R = mybir.dt.float32r
AF = mybir.ActivationFunctionType
OP = mybir.AluOpType


def _r(ap):
    return ap.bitcast(F32R)


def build_nc():
    nc = bacc.Bacc()
    xT = nc.dram_tensor("xT", [H, S], F32, kind="ExternalInput")
    xres = nc.dram_tensor("xres", [SQ, H], F32, kind="ExternalInput")
    wqT = nc.dram_tensor("wqT", [H, H], F32, kind="ExternalInput")
    wkT = nc.dram_tensor("wkT", [H, H], F32, kind="ExternalInput")
    wvT = nc.dram_tensor("wvT", [H, H], F32, kind="ExternalInput")
    woT = nc.dram_tensor("woT", [H, H], F32, kind="ExternalInput")
    bq = nc.dram_tensor("bq", [H], F32, kind="ExternalInput")
    bk = nc.dram_tensor("bk", [H], F32, kind="ExternalInput")
    bv = nc.dram_tensor("bv", [H], F32, kind="ExternalInput")
    bo = nc.dram_tensor("bo", [H], F32, kind="ExternalInput")
    out = nc.dram_tensor("out", [SQ, H], F32, kind="ExternalOutput")

    xT_t = xT[:, :].rearrange("(t p) q -> p t q", p=128)       # [128, 8, 2048]
    wqT_t = wqT[:, :].rearrange("(t p) d -> p t d", p=128)
    wkT_t = wkT[:, :].rearrange("(t p) d -> p t d", p=128)
    wvT_t = wvT[:, :].rearrange("(t p) d -> p t d", p=128)
    woT_t = woT[:, :].rearrange("(t p) d -> p t d", p=128)

    def colvec(v):  # [H] dram -> [128, HT] sbuf layout source AP
        return v[:].rearrange("(t p) -> p t", p=128)

    def bcast_ap(v, parts=128):  # [H] dram -> [parts, H] partition-broadcast AP
        vap = v[:]
        return bass.AP(tensor=vap.tensor, offset=vap.offset,
                       ap=[[0, parts]] + [list(d) for d in vap.ap])

    def pbcast(dram_tile, parts):
        """Partition-broadcast read AP for a [1, N] DRAM pool tile."""
        return bass.AP(tensor=dram_tile.tensor, offset=dram_tile.offset,
                       ap=[[0, parts]] + [list(d) for d in dram_tile.ap[1:]])

    with tile.TileContext(nc) as tc, ExitStack() as ctx:
        persist = ctx.enter_context(tc.tile_pool(name="persist", bufs=1))
        dscratch = ctx.enter_context(tc.tile_pool(name="dscratch", bufs=2, space="DRAM"))

        # ---- persistent sbuf ----
        ynT = persist.tile([128, HT, S], F32R)   # raw x -> normalized x (in place)
        rstd_bc = persist.tile([128, S], F32)
        ms_bc = persist.tile([128, S], F32)
        bqcol = persist.tile([128, HT], F32)
        bkcol = persist.tile([128, HT], F32)
        ones128 = persist.tile([128, 1], F32R)
        ones_f32 = persist.tile([128, 128], F32)

        nc.sync.dma_start(out=bqcol, in_=colvec(bq))
        nc.sync.dma_start(out=bkcol, in_=colvec(bk))
        nc.vector.memset(ones_f32, 1.0)
        nc.vector.tensor_copy(out=ones128, in_=ones_f32[:, 0:1])

        # ---- phase 0+1: LN stats and ynT (scoped pools die early) ----
        with tc.tile_pool(name="p01psum", bufs=1, space="PSUM") as pps, \
                tc.tile_pool(name="phase01", bufs=1) as p01:
            eps_t = p01.tile([1, 1], F32)
            nc.vector.memset(eps_t, 1e-5)
            tldummy = p01.tile([1, 1], F32)
            nc.scalar.sqrt(out=tldummy, in_=eps_t)  # pull sqrt table load early
            sq_v = p01.tile([1, S], F32)
            # single-partition scratch aliased into the (later-written) bc tiles
            sum_v = ms_bc[0:1, :]
            tmp_v = rstd_bc[0:1, :]

            sacc = pps.tile([1, 4, 512], F32, name="sacc")
            qacc = pps.tile([1, 4, 512], F32, name="qacc")
            for h in range(HT):
                xTh = p01.tile([128, S], F32, tag="xTh", bufs=2, name="xTh")
                dma_eng = (nc.sync, nc.gpsimd, nc.scalar)[h % 3]
                dma_eng.dma_start(out=xTh, in_=xT_t[:, h, :])
                nc.vector.tensor_copy(out=ynT[:, h, :], in_=xTh)  # f32r rounding
                xsqh = p01.tile([128, S], F32R, tag="xsq", bufs=2, name="xsqh")
                nc.scalar.square(out=xsqh, in_=xTh)
                for c in range(4):
                    nc.tensor.matmul(sacc[:, c, :], ones128,
                                     ynT[:, h, c * 512:(c + 1) * 512],
                                     start=(h == 0), stop=(h == HT - 1))
                    nc.tensor.matmul(qacc[:, c, :], ones128,
                                     xsqh[:, c * 512:(c + 1) * 512],
                                     start=(h == 0), stop=(h == HT - 1))
            for c in range(4):
                nc.vector.tensor_copy(out=sum_v[:, c * 512:(c + 1) * 512],
                                      in_=sacc[:, c, :])
                nc.vector.tensor_copy(out=sq_v[:, c * 512:(c + 1) * 512],
                                      in_=qacc[:, c, :])

            inv_h = 1.0 / H
            nc.vector.scalar_tensor_tensor(out=tmp_v, in0=sum_v,
                                           scalar=-inv_h * inv_h, in1=sum_v,
                                           op0=OP.mult, op1=OP.mult)  # -mu^2
            nc.vector.scalar_tensor_tensor(out=tmp_v, in0=sq_v, scalar=inv_h,
                                           in1=tmp_v, op0=OP.mult, op1=OP.add)  # var
            nc.scalar.activation(out=tmp_v, in_=tmp_v, func=AF.Sqrt, bias=eps_t[:])
            nc.vector.reciprocal(out=sq_v, in_=tmp_v)             # rstd
            nc.vector.scalar_tensor_tensor(out=tmp_v, in0=sum_v, scalar=-inv_h,
                                           in1=sq_v, op0=OP.mult, op1=OP.mult)  # -mu*rstd
            rstd_d = dscratch.tile([1, S], F32, tag="statd", name="rstd_d")
            ms_d = dscratch.tile([1, S], F32, tag="statd", name="ms_d")
            nc.scalar.dma_start(out=rstd_d, in_=sq_v)
            nc.gpsimd.dma_start(out=ms_d, in_=tmp_v)
            # quartered broadcast reads across queues: quarter-0 normalize
            # unblocks after 256KB instead of 2MB
            for c in range(4):
                sl = slice(c * 512, (c + 1) * 512)
                engr = (nc.scalar, nc.sync)[c % 2]
                engm = (nc.gpsimd, nc.gpsimd)[c % 2]
                engr.dma_start(out=rstd_bc[:, sl], in_=pbcast(rstd_d[0:1, sl], 128))
                engm.dma_start(out=ms_bc[:, sl], in_=pbcast(ms_d[0:1, sl], 128))

        psum = ctx.enter_context(tc.tile_pool(name="psum", bufs=2, space="PSUM"))

        # ---- streaming pool for the rest ----
        stream = ctx.enter_context(tc.tile_pool(name="stream", bufs=1))
        v3 = stream.tile([128, KCH, 8 * 65], F32R)     # V half (8 heads x (64|1))
        qt3 = stream.tile([128, PAIRS, SQ], F32R)      # Q^T per head-pair
        ctxT = stream.tile([128, HT, SQ], F32R)        # attention context^T
        bvcol = stream.tile([128, HT], F32)
        nc.sync.dma_start(out=bvcol, in_=colvec(bv))
        v4 = v3.rearrange("p k (j c) -> p k j c", c=65)
        nc.vector.tensor_copy(
            out=v4[:, :, :, 64:65],
            in_=ones_f32[:, :].rearrange("p (k j) -> p k j", j=8)[:, :, :, None])

        def qt_production():
            for t in range(PAIRS):
                wq_t = stream.tile([128, HT, 128], F32R, tag="wqk", bufs=2,
                                   name="wq_t")
                eng = nc.sync if t % 2 == 0 else nc.gpsimd
                eng.dma_start(out=wq_t,
                              in_=_r(wqT_t[:, :, t * 128:(t + 1) * 128]))
                acc = psum.tile([128, 512], F32, tag="acc", name="acc_q")
                for h in range(HT):
                    nc.tensor.matmul(acc, wq_t[:, h, :], ynT[:, h, 0:SQ],
                                     start=(h == 0), stop=(h == HT - 1))
                nc.scalar.add(out=qt3[:, t, :], in_=acc, add=bqcol[:, t:t + 1])

        def load_wv(hv):
            wv_h = stream.tile([128, HT, 512], F32R, tag="wvwo", name="wv_h")
            nc.sync.dma_start(out=wv_h,
                              in_=_r(wvT_t[:, :, hv * 512:(hv + 1) * 512]))
            return wv_h

        def v_chunk(hv, wv_h, kc):
            acc = psum.tile([128, 512], F32, tag="acc", name="acc_v")
            for h in range(HT):
                nc.tensor.matmul(acc, ynT[:, h, kc * 128:(kc + 1) * 128],
                                 wv_h[:, h, :],
                                 start=(h == 0), stop=(h == HT - 1))
            nc.scalar.copy(out=v4[:, kc, :, 0:64],
                           in_=acc.rearrange("p (j c) -> p j c", c=64))

        # normalize ynT in place per column quarter, interleaved with the
        # projections that quarter unblocks (Q^T + V chunks)

        def pair_begin(t):
            wk_t = stream.tile([128, HT, 128], F32R, tag="wqk", bufs=2, name="wk_t")
            nc.gpsimd.dma_start(out=wk_t,
                                in_=_r(wkT_t[:, :, t * 128:(t + 1) * 128]))
            kt = stream.tile([128, S], F32R, tag="kt", bufs=2, name="kt")
            cpsA = psum.tile([65, 512], F32, tag="ctx", name="cpsA")
            cpsB = psum.tile([65, 512], F32, tag="ctx", name="cpsB")
            return (t, wk_t, kt, cpsA, cpsB)

        def pair_quarter(st, q4):
            t, wk_t, kt, cpsA, cpsB = st
            c4 = q4
            acc = psum.tile([128, 512], F32, tag="acc", name="acc_k")
            for h in range(HT):
                nc.tensor.matmul(acc, wk_t[:, h, :],
                                 ynT[:, h, c4 * 512:(c4 + 1) * 512],
                                 start=(h == 0), stop=(h == HT - 1))
            nc.vector.tensor_scalar_add(kt[:, c4 * 512:(c4 + 1) * 512], acc,
                                        bkcol[:, t:t + 1])
            jA, jB = (2 * t) % 8, (2 * t) % 8 + 1
            for kc in range(4 * q4, 4 * q4 + 4):
                region = psum.tile([128, 1024], F32, tag="region", name="region")
                nc.tensor.matmul(region[:, 0:512],
                                 kt[0:64, kc * 128:(kc + 1) * 128],
                                 qt3[0:64, t, :], start=True, stop=True)
                nc.tensor.matmul(region[:, 512:1024],
                                 kt[64:128, kc * 128:(kc + 1) * 128],
                                 qt3[64:128, t, :], start=True, stop=True)
                et = stream.tile([128, 1024], F32R, tag="expT", bufs=2, name="et")
                nc.scalar.activation(out=et, in_=region, func=AF.Exp, scale=0.125)
                nc.tensor.matmul(cpsA, v3[:, kc, jA * 65:jA * 65 + 65],
                                 et[:, 0:512],
                                 start=(kc == 0), stop=(kc == KCH - 1))
                nc.tensor.matmul(cpsB, v3[:, kc, jB * 65:jB * 65 + 65],
                                 et[:, 512:1024],
                                 start=(kc == 0), stop=(kc == KCH - 1))

        def pair_end(st):
            t, wk_t, kt, cpsA, cpsB = st
            for hh, cps in ((2 * t, cpsA), (2 * t + 1, cpsB)):
                ct = stream.tile([65, 512], F32, tag="ctmp", bufs=2, name="ct")
                nc.vector.tensor_copy(out=ct, in_=cps)
                recip = stream.tile([1, 512], F32, tag="recip", bufs=1, name="recip")
                nc.vector.reciprocal(out=recip, in_=ct[64:65, :])
                rbc = stream.tile([64, 512], F32, tag="rbc", bufs=1, name="rbc")
                nc.gpsimd.partition_broadcast(rbc, recip)
                po = (hh % 2) * 64
                nc.vector.tensor_mul(ctxT[po:po + 64, hh // 2, :], ct[0:64, :], rbc)
                nc.vector.tensor_scalar_add(
                    ctxT[po:po + 64, hh // 2, :], ctxT[po:po + 64, hh // 2, :],
                    bvcol[po:po + 64, hh // 2:hh // 2 + 1])

        def do_pair(t):
            st = pair_begin(t)
            for q4 in range(4):
                pair_quarter(st, q4)
            pair_end(st)

        wv_h0 = load_wv(0)
        st0 = None
        for q4 in range(4):
            for h in range(HT):
                sl = slice(q4 * 512, (q4 + 1) * 512)
                nc.vector.tensor_mul(ynT[:, h, sl], ynT[:, h, sl], rstd_bc[:, sl])
                nc.vector.tensor_add(ynT[:, h, sl], ynT[:, h, sl], ms_bc[:, sl])
            for kc in range(4 * q4, 4 * q4 + 4):
                v_chunk(0, wv_h0, kc)
            if q4 == 0:
                qt_production()
                st0 = pair_begin(0)
            pair_quarter(st0, q4)
        pair_end(st0)

        for t in range(1, 4):
            do_pair(t)
        wv_h1 = load_wv(1)
        for kc in range(KCH):
            v_chunk(1, wv_h1, kc)
        for t in range(4, 8):
            do_pair(t)

        # ---- output projection + bias + residual ----
        bobc = stream.tile([128, H], F32, tag="wvwo", name="bobc")
        nc.gpsimd.dma_start(out=bobc, in_=bcast_ap(bo))
        xres_t = xres[:, :].rearrange("(t p) d -> t p d", p=128)
        for ccq in range(4):
            wo_q = stream.tile([128, HT, 256], F32R, tag="kt", bufs=2, name="wo_q")
            nc.sync.dma_start(out=wo_q,
                              in_=_r(woT_t[:, :, ccq * 256:(ccq + 1) * 256]))
            for qc in range(4):
                acc = psum.tile([128, 256], F32, tag="acc", name="acc_o")
                for h in range(HT):
                    nc.tensor.matmul(acc, ctxT[:, h, qc * 128:(qc + 1) * 128],
                                     wo_q[:, h, :],
                                     start=(h == 0), stop=(h == HT - 1))
                xr = stream.tile([128, 256], F32, tag="xr", bufs=2, name="xr")
                nc.sync.dma_start(out=xr,
                                  in_=xres_t[qc, :, ccq * 256:(ccq + 1) * 256])
                osb = stream.tile([128, 256], F32, tag="osb", bufs=2, name="osb")
                nc.vector.tensor_add(osb, acc, xr)
                nc.vector.tensor_add(osb, osb, bobc[:, ccq * 256:(ccq + 1) * 256])
                nc.sync.dma_start(
                    out=out[qc * 128:(qc + 1) * 128, ccq * 256:(ccq + 1) * 256],
                    in_=osb)
    nc.finalize()
    return nc


_NC = None


def _get_nc():
    global _NC
    if _NC is None:
        _NC = build_nc()
    return _NC


def make_in_maps(inputs):
    x = np.asarray(inputs["x"], np.float32)
    g = np.asarray(inputs["ln_g"], np.float32)
    lnb = np.asarray(inputs["ln_b"], np.float32)
    wq = np.asarray(inputs["Wq"], np.float32)
    wk = np.asarray(inputs["Wk"], np.float32)
    wv = np.asarray(inputs["Wv"], np.float32)
    wo = np.asarray(inputs["Wo"], np.float32)
    # Fold LN affine (gamma/beta) into the QKV weights/biases (exact algebra):
    # xn = y*g + b  =>  xn @ W.T = y @ (W*g).T + (W @ b)
    shared = {
        "wqT": np.ascontiguousarray((wq * g).T),
        "wkT": np.ascontiguousarray((wk * g).T),
        "wvT": np.ascontiguousarray((wv * g).T),
        "woT": np.ascontiguousarray(wo.T),
        "bq": np.asarray(inputs["bq"], np.float32) + wq @ lnb,
        "bk": np.asarray(inputs["bk"], np.float32) + wk @ lnb,
        "bv": np.asarray(inputs["bv"], np.float32) + wv @ lnb,
        "bo": np.asarray(inputs["bo"], np.float32),
    }
    in_maps = []
    for c in range(NCORES):
        b, q0 = c // 4, (c % 4) * SQ
        xbT = x[b].T  # [H, S]
        m = dict(shared)
        # roll so this core's own 512 query columns come first (the kernel is
        # SPMD: one program, per-core data). Attention is invariant to a
        # consistent permutation of the key/value axis.
        m["xT"] = np.ascontiguousarray(np.roll(xbT, -q0, axis=1))
        m["xres"] = np.ascontiguousarray(x[b, q0:q0 + SQ, :])
        in_maps.append(m)
    return in_maps


def kernel(**inputs):
    from concourse.bass_utils import run_bass_kernel_spmd
    nc = _get_nc()
    in_maps = make_in_maps(inputs)
    res = run_bass_kernel_spmd(nc, in_maps, list(range(NCORES)))
    x = np.asarray(inputs["x"], np.float32)
    out = np.empty_like(x)
    for c in range(NCORES):
        b, q0 = c // 4, (c % 4) * SQ
        out[b, q0:q0 + SQ, :] = res.results[c]["out"]
    return out



# revision 12
# speedup vs baseline: 1.5884x; 1.5884x over previous
"""Trainium2 Bass kernel for pre-LN multi-head attention (B=2, S=2048, H=1024, 16 heads).

Sharding: 8 cores = 2 batches x 4 query-blocks of 512 rows (no collectives).
All matmuls run in fp8e4m3 with DoubleRow perf mode (2 contraction tiles per
pass). LayerNorm is folded as ynT8 = fp8(x * rstd) plus a rank-1 correction
ride-along: contraction tile 8 of ynT8 holds ms = -mu*rstd*SM in partition 0,
and row 1024 of each (prescaled) weight matrix holds colsum(W8)/SM, so the
-mu part of LN is applied inside the projection matmuls. Weight matrices are
prescaled by WS=512 on the host (fp8 range), un-scaled at PSUM evacuation.
Softmax: exp on the Act engine over [128,1024] PSUM score regions (fp8 out),
denominator via an appended ones column on V, divided at context evacuation.
"""

import sys
import numpy as np
from contextlib import ExitStack

sys.path.insert(0, "/opt/trn_rl_repo")

import ml_dtypes  # noqa: E402
import concourse.bass as bass  # noqa: E402
import concourse.bacc as bacc  # noqa: E402
import concourse.tile as tile  # noqa: E402
from concourse import mybir  # noqa: E402

B, S, H = 2, 2048, 1024
HEADS, HD = 16, 64
NCORES = 8
SQ = 512          # query rows per core
HT = H // 128     # 8 hidden tiles
PAIRS = HEADS // 2
KCH = S // 128    # 16 key chunks of 128
WS = 512.0        # weight prescale (power of two, exact)
SM = 64.0         # correction-row scale split
F32 = mybir.dt.float32
F32R = mybir.dt.float32r
F8 = mybir.dt.float8e4
AF = mybir.ActivationFunctionType
OP = mybir.AluOpType
DRM = mybir.MatmulPerfMode.DoubleRow
E4 = ml_dtypes.float8_e4m3


def _r(ap):
    return ap.bitcast(F32R)


def slot0(ap):
    """[p, n] AP -> [p, 2, n] AP with stride-0 slot dim (reads data twice)."""
    return bass.AP(tensor=ap.tensor, offset=ap.offset,
                   ap=[list(ap.ap[0])] + [[0, 2]] + [list(d) for d in ap.ap[1:]])


def build_nc():
    nc = bacc.Bacc(num_swdge_queues=2)
    xT = nc.dram_tensor("xT", [H, S], F32R, kind="ExternalInput")
    xrb = nc.dram_tensor("xrb", [SQ, H], F32, kind="ExternalInput")
    wq8 = nc.dram_tensor("wq8", [1280, H], F8, kind="ExternalInput")
    wk8 = nc.dram_tensor("wk8", [1280, H], F8, kind="ExternalInput")
    wv8 = nc.dram_tensor("wv8", [1280, H], F8, kind="ExternalInput")
    wo8 = nc.dram_tensor("wo8", [H, H], F8, kind="ExternalInput")
    bq = nc.dram_tensor("bq", [H], F32, kind="ExternalInput")
    bk = nc.dram_tensor("bk", [H], F32, kind="ExternalInput")
    bv = nc.dram_tensor("bv", [H], F32, kind="ExternalInput")
    out = nc.dram_tensor("out", [SQ, H], F32, kind="ExternalOutput")

    xT_t = xT[:, :].rearrange("(t p) s -> p t s", p=128)        # [128, 8, 2048]
    wq8_t = wq8[:, :].rearrange("(t p) d -> p t d", p=128)      # [128, 10, 1024]
    wk8_t = wk8[:, :].rearrange("(t p) d -> p t d", p=128)
    wv8_t = wv8[:, :].rearrange("(t p) d -> p t d", p=128)
    wo8_t = wo8[:, :].rearrange("(t p) d -> p t d", p=128)      # [128, 8, 1024]
    xrb_t = xrb[:, :].rearrange("(t p) d -> t p d", p=128)

    def colvec(v):  # [H] dram -> [128, HT] sbuf col layout
        return v[:].rearrange("(t p) -> p t", p=128)

    inv_h = 1.0 / H

    with tile.TileContext(nc) as tc, ExitStack() as ctx:
        persist = ctx.enter_context(tc.tile_pool(name="persist", bufs=1))
        stream = ctx.enter_context(tc.tile_pool(name="stream", bufs=1))
        psum = ctx.enter_context(tc.tile_pool(name="psum", bufs=2, space="PSUM"))

        # ---- persistent sbuf ----
        ynT8 = persist.tile([128, 10, S], F8)       # fp8 x*rstd; tile8=ms row; tile9=0
        rstd_bc = persist.tile([128, S], F32)
        qt8 = persist.tile([128, PAIRS, SQ], F8)    # Q^T (pair-tiled)
        v8 = persist.tile([128, KCH, HEADS * 65], F8)   # V rows + ones col per head
        ctxT8 = persist.tile([128, HT, SQ], F8)
        kt8 = [persist.tile([128, 2, S], F8, name=f"kt8_{i}") for i in range(2)]
        wq8s = persist.tile([128, 10, H], F8)
        wk8s = persist.tile([128, 10, H], F8)
        wv8s = persist.tile([128, 10, H], F8)
        wo8s = persist.tile([128, HT, H], F8)
        bqcol = persist.tile([128, HT], F32)
        bkcol = persist.tile([128, HT], F32)
        bv_row = persist.tile([1, H], F32)
        bv_bc = persist.tile([128, H], F32)
        ones128 = persist.tile([128, 1], F32R)
        ones_f32 = persist.tile([128, 1], F32)
        eps_t = persist.tile([1, 1], F32)
        dummy = persist.tile([1, 1], F32)

        # ---- setup DMAs + memsets ----
        nc.gpsimd.dma_start(out=wq8s, in_=wq8_t)
        nc.gpsimd.dma_start(out=wk8s, in_=wk8_t)
        nc.gpsimd.dma_start(out=wv8s, in_=wv8_t)
        nc.gpsimd.dma_start(out=wo8s, in_=wo8_t)
        nc.sync.dma_start(out=bqcol, in_=colvec(bq))
        nc.sync.dma_start(out=bkcol, in_=colvec(bk))
        nc.scalar.dma_start(out=bv_row, in_=bv[:].rearrange("(o d) -> o d", o=1))
        nc.gpsimd.partition_broadcast(bv_bc, bv_row)
        nc.vector.memset(ones_f32, 1.0)
        nc.vector.tensor_copy(out=ones128, in_=ones_f32)
        nc.vector.memset(eps_t, 1e-5)
        nc.scalar.sqrt(out=dummy, in_=eps_t)    # pull Sqrt table load early
        nc.gpsimd.memset(kt8[0][:, 1, :], 0.0)  # DR slot-1 zeros (stay zero)
        nc.gpsimd.memset(kt8[1][:, 1, :], 0.0)
        nc.gpsimd.memset(ynT8[:, 8, :], 0.0)   # ms row: epilogue rewrites part 0
        nc.gpsimd.memset(ynT8[:, 9, :], 0.0)
        # ones columns of V (denominator trick): v8[:, kc, j*65+64] = 1
        v8_j = v8.rearrange("p k (j c) -> p k j c", c=65)
        nc.gpsimd.memset(v8_j[:, :, :, 64:65], 1.0)

        # ---- phase 0: stats + prep, pipelined by column quarter ----
        xq = {}

        def quarter(q, spool):
            sl = slice(q * 512, (q + 1) * 512)
            for h in range(HT):
                t_ = stream.tile([128, 512], F32R, tag="xq", bufs=16, name="xq")
                eng = nc.sync if h % 2 == 0 else nc.scalar
                eng.dma_start(out=t_, in_=xT_t[:, h, sl])
                xq[(q, h)] = t_
            sacc = spool.tile([1, 512], F32, tag="sacc", name="sacc")
            qacc = spool.tile([1, 512], F32, tag="qacc", name="qacc")
            for h in range(HT):
                t_ = xq[(q, h)]
                xsq = stream.tile([128, 512], F32R, tag="xsq", bufs=2, name="xsq")
                nc.gpsimd.tensor_mul(xsq, t_, t_)
                nc.tensor.matmul(sacc, ones128, t_[:, :],
                                 start=(h == 0), stop=(h == HT - 1))
                nc.tensor.matmul(qacc, ones128, xsq,
                                 start=(h == 0), stop=(h == HT - 1))
            tmp = stream.tile([1, 512], F32, tag="tmp", bufs=2, name="tmp")
            var = stream.tile([1, 512], F32, tag="var", bufs=2, name="var")
            rstd = stream.tile([1, 512], F32, tag="rstd", bufs=2, name="rstd")
            srow = stream.tile([1, 512], F32, tag="srow", bufs=2, name="srow")
            nc.vector.tensor_copy(out=srow, in_=sacc)
            nc.vector.scalar_tensor_tensor(out=tmp, in0=sacc, scalar=-inv_h * inv_h,
                                           in1=srow, op0=OP.mult, op1=OP.mult)
            nc.vector.scalar_tensor_tensor(out=var, in0=qacc, scalar=inv_h,
                                           in1=tmp, op0=OP.mult, op1=OP.add)
            nc.scalar.activation(out=var, in_=var, func=AF.Sqrt, bias=eps_t[:])
            nc.vector.reciprocal(out=rstd, in_=var)
            # ms row (fp8): -mu * rstd * SM  -> ynT8 tile 8, partition 0
            nc.vector.scalar_tensor_tensor(out=ynT8[0:1, 8, sl], in0=sacc,
                                           scalar=-inv_h * SM, in1=rstd,
                                           op0=OP.mult, op1=OP.mult)
            nc.gpsimd.partition_broadcast(rstd_bc[:, sl], rstd)
            # prep: ynT8 = fp8(x * rstd); q0 on DVE for a fast pipeline start
            peng = nc.vector if q == 0 else nc.gpsimd
            for h in range(HT):
                peng.tensor_mul(ynT8[:, h, sl], xq[(q, h)], rstd_bc[:, sl])

        def proj_group(w8s, t, movsl, dst_kind):
            """5 DoubleRow steps; movsl = column slice of ynT8 (as moving for
            q/k) or of the w (as moving for v/o).  dst_kind picks operand roles."""
            acc = psum.tile([128, 512], F32, tag="acc", name="acc")
            if dst_kind == "qk":   # out [128 dims, 512 cols]; moving = ynT8
                for i in range(5):
                    nc.tensor.matmul(acc[:, 0:movsl.stop - movsl.start],
                                     w8s[:, 2 * i:2 * i + 2, t * 128:(t + 1) * 128],
                                     ynT8[:, 2 * i:2 * i + 2, movsl],
                                     start=(i == 0), stop=(i == 4), perf_mode=DRM)
            else:                  # "vo": out [128 keys/q, 512 dims]; moving = W
                pass
            return acc

        # --- emission schedule ---
        with tc.tile_pool(name="statps", bufs=2, space="PSUM") as spool:
            quarter(0, spool)
            quarter(1, spool)

            # Q projection (own 512 query columns)
            for t in range(PAIRS):
                acc = proj_group(wq8s, t, slice(0, 512), "qk")
                nc.vector.tensor_scalar(out=qt8[:, t, :], in0=acc,
                                        scalar1=1.0 / WS, scalar2=bqcol[:, t:t + 1],
                                        op0=OP.mult, op1=OP.add)

            quarter(2, spool)

            def v_group(kc, jh):
                acc = psum.tile([128, 512], F32, tag="acc", name="acc_v")
                ksl = slice(kc * 128, (kc + 1) * 128)
                dsl = slice(jh * 512, (jh + 1) * 512)
                for i in range(5):
                    nc.tensor.matmul(acc, ynT8[:, 2 * i:2 * i + 2, ksl],
                                     wv8s[:, 2 * i:2 * i + 2, dsl],
                                     start=(i == 0), stop=(i == 4), perf_mode=DRM)
                # evac: (psum/WS) + bv -> v8 (8 heads x 64 dims, stride 65)
                nc.vector.scalar_tensor_tensor(
                    out=v8_j[:, kc, 8 * jh:8 * jh + 8, 0:64], in0=acc,
                    scalar=1.0 / WS, in1=bv_bc[:, dsl], op0=OP.mult, op1=OP.add)

            def k_group(pair, q, kbuf):
                sl = slice(q * 512, (q + 1) * 512)
                acc = proj_group(wk8s, pair, sl, "qk")
                nc.vector.tensor_scalar(out=kt8[kbuf][:, 0, sl], in0=acc,
                                        scalar1=1.0 / WS,
                                        scalar2=bkcol[:, pair:pair + 1],
                                        op0=OP.mult, op1=OP.add)

            for kc in range(0, 4):
                v_group(kc, 0)
                v_group(kc, 1)
            k_group(0, 0, 0)
            k_group(0, 1, 0)

            quarter(3, spool)

            for kc in range(4, 8):
                v_group(kc, 0)
                v_group(kc, 1)
            k_group(0, 2, 0)
            for kc in range(8, 12):
                v_group(kc, 0)
                v_group(kc, 1)
            k_group(0, 3, 0)
            for kc in range(12, 16):
                v_group(kc, 0)
                v_group(kc, 1)

        # warm the Exp table before the storm
        nc.scalar.activation(out=dummy, in_=eps_t, func=AF.Exp)

        with tc.tile_pool(name="regpool", bufs=2, space="PSUM") as rpool:

            def head(j, pair, kbuf, kwork):
                po = 64 * (j % 2)
                cps = psum.tile([65, 512], F32, tag="ctx", name="cps")
                qmov = slot0(qt8[po:po + 64, pair, :])
                for reg in range(8):
                    kc0 = 2 * reg
                    region = rpool.tile([128, 1024], F32, tag="region", name="reg")
                    nc.tensor.matmul(
                        region[:, 0:512],
                        kt8[kbuf][po:po + 64, :, kc0 * 128:(kc0 + 1) * 128],
                        qmov, start=True, stop=True, perf_mode=DRM)
                    nc.tensor.matmul(
                        region[:, 512:1024],
                        kt8[kbuf][po:po + 64, :, (kc0 + 1) * 128:(kc0 + 2) * 128],
                        qmov, start=True, stop=True, perf_mode=DRM)
                    et = stream.tile([128, 2, 512], F8, tag="et", bufs=2, name="et")
                    nc.scalar.activation(out=et, in_=region, func=AF.Exp,
                                         scale=0.125)
                    nc.tensor.matmul(cps, v8[:, kc0:kc0 + 2, j * 65:j * 65 + 65],
                                     et, start=(reg == 0), stop=(reg == 7),
                                     perf_mode=DRM)
                    if reg in (1, 3) and kwork:
                        kwork.pop(0)()
                # evac: ctxT8 = fp8(cps[0:64] * (1/den))
                recip = stream.tile([1, 512], F32, tag="recip", bufs=2, name="recip")
                nc.vector.reciprocal(out=recip, in_=cps[64:65, :])
                rbc = stream.tile([64, 512], F32, tag="rbc", bufs=2, name="rbc")
                nc.gpsimd.partition_broadcast(rbc, recip)
                nc.vector.tensor_mul(ctxT8[po:po + 64, pair, :], cps[0:64, :], rbc)

            for pair in range(PAIRS):
                kbuf = pair % 2
                nbuf = (pair + 1) % 2
                if pair < PAIRS - 1:
                    kw = [lambda q=q: k_group(pair + 1, q, nbuf) for q in range(4)]
                else:
                    kw = []
                head(2 * pair, pair, kbuf, kw)
                head(2 * pair + 1, pair, kbuf, kw)

            # ---- output projection + residual ----
            for qc in range(4):
                for jh in range(2):
                    dsl = slice(jh * 512, (jh + 1) * 512)
                    acc = psum.tile([128, 512], F32, tag="acc", name="acc_o")
                    for i in range(4):
                        nc.tensor.matmul(acc, ctxT8[:, 2 * i:2 * i + 2,
                                                    qc * 128:(qc + 1) * 128],
                                         wo8s[:, 2 * i:2 * i + 2, dsl],
                                         start=(i == 0), stop=(i == 3),
                                         perf_mode=DRM)
                    xr = stream.tile([128, 512], F32, tag="xr", bufs=2, name="xr")
                    nc.sync.dma_start(out=xr, in_=xrb_t[qc, :, dsl])
                    osb = stream.tile([128, 512], F32, tag="osb", bufs=2, name="osb")
                    nc.vector.scalar_tensor_tensor(out=osb, in0=acc, scalar=1.0 / WS,
                                                   in1=xr, op0=OP.mult, op1=OP.add)
                    nc.sync.dma_start(
                        out=out[qc * 128:(qc + 1) * 128, dsl], in_=osb)
    nc.finalize()
    return nc


_NC = None


def _get_nc():
    global _NC
    if _NC is None:
        _NC = build_nc()
    return _NC


def _q8(a):
    return np.asarray(a, np.float32).astype(E4)


def make_in_maps(inputs):
    x = np.asarray(inputs["x"], np.float32)
    g = np.asarray(inputs["ln_g"], np.float32)
    lnb = np.asarray(inputs["ln_b"], np.float32)
    wq = np.asarray(inputs["Wq"], np.float32)
    wk = np.asarray(inputs["Wk"], np.float32)
    wv = np.asarray(inputs["Wv"], np.float32)
    wo = np.asarray(inputs["Wo"], np.float32)

    def prep_w(w):
        """[H,H] torch-layout W -> [1280,H] fp8: rows 0-1023 = fp8(WS*(W*g).T),
        row 1024 = fp8(colsum/SM), rest zero."""
        w8 = _q8(WS * (w * g).T)
        full = np.zeros((1280, H), E4)
        full[0:H] = w8
        full[H] = _q8(w8.astype(np.float32).sum(0) / SM)
        return full

    shared = {
        "wq8": prep_w(wq),
        "wk8": prep_w(wk),
        "wv8": prep_w(wv),
        "wo8": _q8(WS * wo.T),
        "bq": np.asarray(inputs["bq"], np.float32) + wq @ lnb,
        "bk": np.asarray(inputs["bk"], np.float32) + wk @ lnb,
        "bv": np.asarray(inputs["bv"], np.float32) + wv @ lnb,
    }
    bo = np.asarray(inputs["bo"], np.float32)
    in_maps = []
    for c in range(NCORES):
        b, q0 = c // 4, (c % 4) * SQ
        xbT = x[b].T  # [H, S]
        m = dict(shared)
        # roll so this core's own 512 query columns come first (SPMD: one
        # program, per-core data); attention is invariant to a consistent
        # permutation of the key/value axis.
        m["xT"] = np.ascontiguousarray(np.roll(xbT, -q0, axis=1))
        m["xrb"] = x[b, q0:q0 + SQ, :] + bo
        in_maps.append(m)
    return in_maps


def kernel(**inputs):
    from concourse.bass_utils import run_bass_kernel_spmd
    nc = _get_nc()
    in_maps = make_in_maps(inputs)
    res = run_bass_kernel_spmd(nc, in_maps, list(range(NCORES)))
    x = np.asarray(inputs["x"], np.float32)
    out = np.empty_like(x)
    for c in range(NCORES):
        b, q0 = c // 4, (c % 4) * SQ
        out[b, q0:q0 + SQ, :] = res.results[c]["out"]
    return out


# revision 20
# speedup vs baseline: 1.6551x; 1.0420x over previous
"""Trainium2 Bass kernel for pre-LN multi-head attention (B=2, S=2048, H=1024, 16 heads).

Sharding: 8 cores = 2 batches x 4 query-blocks of 512 rows (no collectives).
All matmuls run in fp8e4m3 with DoubleRow perf mode (2 contraction tiles per
pass). LayerNorm is folded as ynT8 = fp8(x * rstd) plus a rank-1 correction
ride-along: contraction tile 8 of ynT8 holds ms = -mu*rstd*SM in partition 0,
and row 1024 of each (prescaled) weight matrix holds colsum(W8)/SM, so the
-mu part of LN is applied inside the projection matmuls. Weight matrices are
prescaled by WS=512 on the host (fp8 range), un-scaled at PSUM evacuation.
Softmax: exp on the Act engine over [128,1024] PSUM score regions (fp8 out),
denominator via an appended ones column on V, divided at context evacuation.
"""

import sys
import numpy as np
from contextlib import ExitStack

sys.path.insert(0, "/opt/trn_rl_repo")

import ml_dtypes  # noqa: E402
import concourse.bass as bass  # noqa: E402
import concourse.bacc as bacc  # noqa: E402
import concourse.tile as tile  # noqa: E402
from concourse import mybir  # noqa: E402

B, S, H = 2, 2048, 1024
HEADS, HD = 16, 64
NCORES = 8
SQ = 512          # query rows per core
HT = H // 128     # 8 hidden tiles
PAIRS = HEADS // 2
KCH = S // 128    # 16 key chunks of 128
WS = 512.0        # weight prescale (power of two, exact)
SM = 64.0         # correction-row scale split
F32 = mybir.dt.float32
F32R = mybir.dt.float32r
F8 = mybir.dt.float8e4
AF = mybir.ActivationFunctionType
OP = mybir.AluOpType
DRM = mybir.MatmulPerfMode.DoubleRow
E4 = ml_dtypes.float8_e4m3


def _r(ap):
    return ap.bitcast(F32R)


def slot0(ap):
    """[p, n] AP -> [p, 2, n] AP with stride-0 slot dim (reads data twice)."""
    return bass.AP(tensor=ap.tensor, offset=ap.offset,
                   ap=[list(ap.ap[0])] + [[0, 2]] + [list(d) for d in ap.ap[1:]])


def build_nc():
    nc = bacc.Bacc(num_swdge_queues=2)
    xT = nc.dram_tensor("xT", [H, S], F32R, kind="ExternalInput")
    xrb = nc.dram_tensor("xrb", [SQ, H], F32, kind="ExternalInput")
    wq8 = nc.dram_tensor("wq8", [1280, H], F8, kind="ExternalInput")
    wk8 = nc.dram_tensor("wk8", [1280, H], F8, kind="ExternalInput")
    wv8 = nc.dram_tensor("wv8", [1280, H], F8, kind="ExternalInput")
    wo8 = nc.dram_tensor("wo8", [H, H], F8, kind="ExternalInput")
    bq = nc.dram_tensor("bq", [H], F32, kind="ExternalInput")
    bk = nc.dram_tensor("bk", [H], F32, kind="ExternalInput")
    bv = nc.dram_tensor("bv", [H], F32, kind="ExternalInput")
    out = nc.dram_tensor("out", [SQ, H], F32, kind="ExternalOutput")

    xT_t = xT[:, :].rearrange("(t p) s -> p t s", p=128)        # [128, 8, 2048]
    wq8_t = wq8[:, :].rearrange("(t p) d -> p t d", p=128)      # [128, 10, 1024]
    wk8_t = wk8[:, :].rearrange("(t p) d -> p t d", p=128)
    wv8_t = wv8[:, :].rearrange("(t p) d -> p t d", p=128)
    wo8_t = wo8[:, :].rearrange("(t p) d -> p t d", p=128)      # [128, 8, 1024]
    xrb_t = xrb[:, :].rearrange("(t p) d -> t p d", p=128)

    def colvec(v):  # [H] dram -> [128, HT] sbuf col layout
        return v[:].rearrange("(t p) -> p t", p=128)

    inv_h = 1.0 / H

    with tile.TileContext(nc) as tc, ExitStack() as ctx:
        persist = ctx.enter_context(tc.tile_pool(name="persist", bufs=1))
        stream = ctx.enter_context(tc.tile_pool(name="stream", bufs=1))
        psum = ctx.enter_context(tc.tile_pool(name="psum", bufs=2, space="PSUM"))

        # ---- persistent sbuf ----
        ynT8 = persist.tile([128, 10, S], F8)       # fp8 x*rstd; tile8=ms row; tile9=0
        rstd_bc = persist.tile([128, S], F32)
        qt8 = persist.tile([128, PAIRS, SQ], F8)    # Q^T (pair-tiled)
        v8 = persist.tile([128, KCH, HEADS * 65], F8)   # V rows + ones col per head
        ctxT8 = persist.tile([128, HT, SQ], F8)
        kt8 = [persist.tile([128, 2, S], F8, name=f"kt8_{i}") for i in range(2)]
        wq8s = persist.tile([128, 10, H], F8)
        wk8s = persist.tile([128, 10, H], F8)
        wv8s = persist.tile([128, 10, H], F8)
        wo8s = persist.tile([128, HT, H], F8)
        bqcol = persist.tile([128, HT], F32)
        bkcol = persist.tile([128, HT], F32)
        bv_row = persist.tile([1, H], F32)
        bv_bc = persist.tile([128, H], F32)
        ones128 = persist.tile([128, 1], F32R)
        ones_f32 = persist.tile([128, 1], F32)
        eps_t = persist.tile([1, 1], F32)
        dummy = persist.tile([1, 1], F32)

        # ---- setup DMAs + memsets ----
        nc.gpsimd.dma_start(out=wq8s, in_=wq8_t)
        nc.gpsimd.dma_start(out=wk8s, in_=wk8_t)
        nc.gpsimd.dma_start(out=wv8s, in_=wv8_t)
        nc.gpsimd.dma_start(out=wo8s, in_=wo8_t)
        nc.sync.dma_start(out=bqcol, in_=colvec(bq))
        nc.sync.dma_start(out=bkcol, in_=colvec(bk))
        nc.scalar.dma_start(out=bv_row, in_=bv[:].rearrange("(o d) -> o d", o=1))
        nc.gpsimd.partition_broadcast(bv_bc, bv_row)
        nc.vector.memset(ones_f32, 1.0)
        nc.vector.tensor_copy(out=ones128, in_=ones_f32)
        nc.vector.memset(eps_t, 1e-5)
        # pull the sqrt/square table load early
        nc.scalar.activation(out=dummy, in_=eps_t, func=AF.Sqrt)
        v8_j = v8.rearrange("p k (j c) -> p k j c", c=65)
        # correction-row slots: tile 8 = ms row (part 0 written per quarter),
        # tile 9 = DoubleRow zero pad. DVE is idle at start.
        nc.vector.memset(ynT8[:, 8, :], 0.0)
        nc.vector.memset(ynT8[:, 9, :], 0.0)

        # ---- phase 0: stats + prep, pipelined by column quarter ----
        xq = {}

        def quarter(q, spool):
            sl = slice(q * 512, (q + 1) * 512)
            # grouped x DMAs: h-even tiles in one transfer (sync), h-odd in
            # another (scalar) - cheap issue, same arrival cadence
            xe = stream.tile([128, 4, 512], F32R, tag="xqe", bufs=2, name="xqe")
            xo = stream.tile([128, 4, 512], F32R, tag="xqo", bufs=2, name="xqo")
            nc.sync.dma_start(out=xe, in_=xT_t[:, 0::2, sl])
            nc.scalar.dma_start(out=xo, in_=xT_t[:, 1::2, sl])
            for h in range(HT):
                xq[(q, h)] = (xe if h % 2 == 0 else xo)[:, h // 2, :]
            sacc = spool.tile([1, 512], F32, tag="sacc", name="sacc")
            qacc = spool.tile([1, 512], F32, tag="qacc", name="qacc")
            for h in range(HT):
                t_ = xq[(q, h)]
                xsq = stream.tile([128, 512], F32R, tag="xsq", bufs=3, name="xsq")
                # split squares between Act (idle pre-softmax) and gpsimd
                if h < 5:
                    nc.scalar.activation(out=xsq, in_=t_, func=AF.Square)
                else:
                    nc.gpsimd.tensor_mul(xsq, t_, t_)
                nc.tensor.matmul(sacc, ones128, t_,
                                 start=(h == 0), stop=(h == HT - 1))
                nc.tensor.matmul(qacc, ones128, xsq,
                                 start=(h == 0), stop=(h == HT - 1))
            tmp = stream.tile([1, 512], F32, tag="tmp", bufs=2, name="tmp")
            var = stream.tile([1, 512], F32, tag="var", bufs=2, name="var")
            rstd = stream.tile([1, 512], F32, tag="rstd", bufs=2, name="rstd")
            srow = stream.tile([1, 512], F32, tag="srow", bufs=2, name="srow")
            nc.vector.tensor_copy(out=srow, in_=sacc)
            nc.vector.scalar_tensor_tensor(out=tmp, in0=sacc, scalar=-inv_h * inv_h,
                                           in1=srow, op0=OP.mult, op1=OP.mult)
            nc.vector.scalar_tensor_tensor(out=var, in0=qacc, scalar=inv_h,
                                           in1=tmp, op0=OP.mult, op1=OP.add)
            nc.scalar.activation(out=var, in_=var, func=AF.Sqrt, bias=eps_t[:])
            nc.vector.reciprocal(out=rstd, in_=var)
            # ms row (fp8): -mu * rstd * SM  -> ynT8 tile 8, partition 0
            nc.vector.scalar_tensor_tensor(out=ynT8[0:1, 8, sl], in0=sacc,
                                           scalar=-inv_h * SM, in1=rstd,
                                           op0=OP.mult, op1=OP.mult)
            nc.gpsimd.partition_broadcast(rstd_bc[:, sl], rstd)
            # prep: ynT8 = fp8(x * rstd); q0 on DVE for a fast pipeline start
            peng = nc.vector if q == 0 else nc.gpsimd
            for h in range(HT):
                peng.tensor_mul(ynT8[:, h, sl], xq[(q, h)], rstd_bc[:, sl])

        def proj_group(w8s, t, movsl, dst_kind):
            """5 DoubleRow steps; movsl = column slice of ynT8 (as moving for
            q/k) or of the w (as moving for v/o).  dst_kind picks operand roles."""
            acc = psum.tile([128, 512], F32, tag="acc", name="acc")
            if dst_kind == "qk":   # out [128 dims, 512 cols]; moving = ynT8
                for i in range(5):
                    nc.tensor.matmul(acc[:, 0:movsl.stop - movsl.start],
                                     w8s[:, 2 * i:2 * i + 2, t * 128:(t + 1) * 128],
                                     ynT8[:, 2 * i:2 * i + 2, movsl],
                                     start=(i == 0), stop=(i == 4), perf_mode=DRM)
            else:                  # "vo": out [128 keys/q, 512 dims]; moving = W
                pass
            return acc

        # --- emission schedule ---
        with tc.tile_pool(name="statps", bufs=2, space="PSUM") as spool:
            quarter(0, spool)
            quarter(1, spool)

            # deferred memsets (keep them off the phase-0 critical path)
            nc.gpsimd.memset(kt8[0][:, 1, :], 0.0)  # DR slot-1 zeros (stay zero)
            nc.gpsimd.memset(kt8[1][:, 1, :], 0.0)
            # ones columns of V (denominator trick): v8[:, kc, j*65+64] = 1
            nc.gpsimd.memset(v8_j[:, :, :, 64:65], 1.0)

            # Q projection (own 512 query columns)
            for t in range(PAIRS):
                acc = proj_group(wq8s, t, slice(0, 512), "qk")
                nc.vector.tensor_scalar(out=qt8[:, t, :], in0=acc,
                                        scalar1=1.0 / WS, scalar2=bqcol[:, t:t + 1],
                                        op0=OP.mult, op1=OP.add)

            quarter(2, spool)

            def v_group(kc, jh):
                acc = psum.tile([128, 512], F32, tag="acc", name="acc_v")
                ksl = slice(kc * 128, (kc + 1) * 128)
                dsl = slice(jh * 512, (jh + 1) * 512)
                for i in range(5):
                    nc.tensor.matmul(acc, ynT8[:, 2 * i:2 * i + 2, ksl],
                                     wv8s[:, 2 * i:2 * i + 2, dsl],
                                     start=(i == 0), stop=(i == 4), perf_mode=DRM)
                # evac: (psum/WS) + bv -> v8 (8 heads x 64 dims, stride 65)
                nc.vector.scalar_tensor_tensor(
                    out=v8_j[:, kc, 8 * jh:8 * jh + 8, 0:64], in0=acc,
                    scalar=1.0 / WS, in1=bv_bc[:, dsl], op0=OP.mult, op1=OP.add)

            def k_group(pair, q, kbuf):
                sl = slice(q * 512, (q + 1) * 512)
                acc = proj_group(wk8s, pair, sl, "qk")
                nc.vector.tensor_scalar(out=kt8[kbuf][:, 0, sl], in0=acc,
                                        scalar1=1.0 / WS,
                                        scalar2=bkcol[:, pair:pair + 1],
                                        op0=OP.mult, op1=OP.add)

            for kc in range(0, 4):
                v_group(kc, 0)
                v_group(kc, 1)
            k_group(0, 0, 0)
            k_group(0, 1, 0)

            quarter(3, spool)

            for kc in range(4, 8):
                v_group(kc, 0)
                v_group(kc, 1)
            k_group(0, 2, 0)
            for kc in range(8, 12):
                v_group(kc, 0)
                v_group(kc, 1)
            k_group(0, 3, 0)
            for kc in range(12, 16):
                v_group(kc, 0)
                v_group(kc, 1)

        # warm the Exp table before the storm
        nc.scalar.activation(out=dummy, in_=eps_t, func=AF.Exp)

        with tc.tile_pool(name="regpool", bufs=2, space="PSUM") as rpool:

            def head(j, pair, kbuf, kwork):
                po = 64 * (j % 2)
                cps = psum.tile([65, 512], F32, tag="ctx", name="cps")
                qmov = slot0(qt8[po:po + 64, pair, :])
                pend_ctx = []   # emit ctx-DR one region late so an in-order PE
                                # stall on cps WAR never blocks the next scores

                def scores_exp(reg):
                    kc0 = 2 * reg
                    region = rpool.tile([128, 1024], F32, tag="region", name="reg")
                    nc.tensor.matmul(
                        region[:, 0:512],
                        kt8[kbuf][po:po + 64, :, kc0 * 128:(kc0 + 1) * 128],
                        qmov, start=True, stop=True, perf_mode=DRM)
                    nc.tensor.matmul(
                        region[:, 512:1024],
                        kt8[kbuf][po:po + 64, :, (kc0 + 1) * 128:(kc0 + 2) * 128],
                        qmov, start=True, stop=True, perf_mode=DRM)
                    et = stream.tile([128, 2, 512], F8, tag="et", bufs=4, name="et")
                    nc.scalar.activation(out=et, in_=region, func=AF.Exp,
                                         scale=0.125)
                    pend_ctx.append((reg, et))

                def ctx_dr():
                    reg, et = pend_ctx.pop(0)
                    kc0 = 2 * reg
                    nc.tensor.matmul(cps, v8[:, kc0:kc0 + 2, j * 65:j * 65 + 65],
                                     et, start=(reg == 0), stop=(reg == 7),
                                     perf_mode=DRM)

                for reg in range(8):
                    scores_exp(reg)
                    if reg >= 1:
                        ctx_dr()
                    if reg in (2, 5) and kwork:
                        kwork.pop(0)()
                ctx_dr()
                # evac: ctxT8 = fp8(cps[0:64] * (1/den))
                recip = stream.tile([1, 512], F32, tag="recip", bufs=2, name="recip")
                nc.vector.reciprocal(out=recip, in_=cps[64:65, :])
                rbc = stream.tile([64, 512], F32, tag="rbc", bufs=2, name="rbc")
                nc.gpsimd.partition_broadcast(rbc, recip)
                nc.vector.tensor_mul(ctxT8[po:po + 64, pair, :], cps[0:64, :], rbc)

            xr_tiles = {}
            for pair in range(PAIRS):
                kbuf = pair % 2
                nbuf = (pair + 1) % 2
                if pair < PAIRS - 1:
                    kw = [lambda q=q: k_group(pair + 1, q, nbuf) for q in range(4)]
                else:
                    kw = []
                head(2 * pair, pair, kbuf, kw)
                head(2 * pair + 1, pair, kbuf, kw)
                if pair == 5:
                    # prefetch residual tiles for the output projection
                    for qc in range(4):
                        for jh in range(2):
                            xr = stream.tile([128, 512], F32, tag="xr", bufs=8,
                                             name="xr")
                            nc.sync.dma_start(
                                out=xr, in_=xrb_t[qc, :, jh * 512:(jh + 1) * 512])
                            xr_tiles[(qc, jh)] = xr

            # ---- output projection + residual ----
            for qc in range(4):
                for jh in range(2):
                    dsl = slice(jh * 512, (jh + 1) * 512)
                    acc = psum.tile([128, 512], F32, tag="acc", name="acc_o")
                    for i in range(4):
                        nc.tensor.matmul(acc, ctxT8[:, 2 * i:2 * i + 2,
                                                    qc * 128:(qc + 1) * 128],
                                         wo8s[:, 2 * i:2 * i + 2, dsl],
                                         start=(i == 0), stop=(i == 3),
                                         perf_mode=DRM)
                    osb = stream.tile([128, 512], F32, tag="osb", bufs=2, name="osb")
                    nc.vector.scalar_tensor_tensor(out=osb, in0=acc, scalar=1.0 / WS,
                                                   in1=xr_tiles[(qc, jh)],
                                                   op0=OP.mult, op1=OP.add)
                    nc.sync.dma_start(
                        out=out[qc * 128:(qc + 1) * 128, dsl], in_=osb)
    nc.finalize()
    return nc


_NC = None


def _get_nc():
    global _NC
    if _NC is None:
        _NC = build_nc()
    return _NC


def _q8(a):
    return np.asarray(a, np.float32).astype(E4)


def make_in_maps(inputs):
    x = np.asarray(inputs["x"], np.float32)
    g = np.asarray(inputs["ln_g"], np.float32)
    lnb = np.asarray(inputs["ln_b"], np.float32)
    wq = np.asarray(inputs["Wq"], np.float32)
    wk = np.asarray(inputs["Wk"], np.float32)
    wv = np.asarray(inputs["Wv"], np.float32)
    wo = np.asarray(inputs["Wo"], np.float32)

    def prep_w(w):
        """[H,H] torch-layout W -> [1280,H] fp8: rows 0-1023 = fp8(WS*(W*g).T),
        row 1024 = fp8(colsum/SM), rest zero."""
        w8 = _q8(WS * (w * g).T)
        full = np.zeros((1280, H), E4)
        full[0:H] = w8
        full[H] = _q8(w8.astype(np.float32).sum(0) / SM)
        return full

    shared = {
        "wq8": prep_w(wq),
        "wk8": prep_w(wk),
        "wv8": prep_w(wv),
        "wo8": _q8(WS * wo.T),
        "bq": np.asarray(inputs["bq"], np.float32) + wq @ lnb,
        "bk": np.asarray(inputs["bk"], np.float32) + wk @ lnb,
        "bv": np.asarray(inputs["bv"], np.float32) + wv @ lnb,
    }
    bo = np.asarray(inputs["bo"], np.float32)
    in_maps = []
    for c in range(NCORES):
        b, q0 = c // 4, (c % 4) * SQ
        xbT = x[b].T  # [H, S]
        m = dict(shared)
        # roll so this core's own 512 query columns come first (SPMD: one
        # program, per-core data); attention is invariant to a consistent
        # permutation of the key/value axis.
        m["xT"] = np.ascontiguousarray(np.roll(xbT, -q0, axis=1))
        m["xrb"] = x[b, q0:q0 + SQ, :] + bo
        in_maps.append(m)
    return in_maps


def kernel(**inputs):
    from concourse.bass_utils import run_bass_kernel_spmd
    nc = _get_nc()
    in_maps = make_in_maps(inputs)
    res = run_bass_kernel_spmd(nc, in_maps, list(range(NCORES)))
    x = np.asarray(inputs["x"], np.float32)
    out = np.empty_like(x)
    for c in range(NCORES):
        b, q0 = c // 4, (c % 4) * SQ
        out[b, q0:q0 + SQ, :] = res.results[c]["out"]
    return out
